# revision 11
# baseline (speedup 1.0000x reference)
"""EnvMap BRDF render-loss kernel for 8 Trainium2 NeuronCores.

Strategy (matches the sharding hint): shard the S=128 sample axis across the
8 cores (16 samples each).  The envmap gather (env[:, idy, idx, :] for the 4
environment images x0,x1,y0,y1) is performed host-side and the gathered
tensors are sharded along S; each core streams its [12 env-channels, 16
samples, 65536 pixels] bf16 slab and computes the full per-(sample,pixel)
BRDF chain + weighted accumulation, producing partial sums [12, 65536] f32.
Host reduces partials over cores, forms pred/gt and the scalar loss.

All math is algebraically identical to the reference:
  camy = normalize(up - (up.n)n);  cross(camy,n) = cross(up,n)/|cy| =
  (nz,0,-nx)/|cy|  =>  camx = (-nz,0,nx)/(|nz|+|nx|)  (L1-normalized).
  n.l = ls_z (basis orthogonality), v.v = 1, |camy| = 1.
  out_{e,c}[pix] = sum_s (alpha_s*diffB_c + beta_s*spec_{s,pix}) * G_{e,c,s,pix}
  with alpha_s = ew_s*ls_z_s, beta_s = 10*ew_s*ls_z_s.
"""

import numpy as np
import ml_dtypes

IM = 256
NPIX = IM * IM          # 65536
EXH, EXW = 128, 256
S = 128
NDEV = 8
SLOC = S // NDEV        # 16 samples per device
P = 128                 # partitions
F = NPIX // P           # 512 free-dim pixels
F0C = 0.05
NCH = 12                # 4 envs x 3 channels
PI = float(np.pi)

_BUILT = None           # cached (nc, meta)


# ----------------------------------------------------------------------------
# Dependency tracker: records ops per engine with explicit semaphore waits.
# ----------------------------------------------------------------------------
class Prog:
    def __init__(self):
        self.ops = {"V": [], "A": [], "S": []}   # per-engine [(waits, emit_fn)]
        self.tick = {"V": 0, "A": 0}
        self.lane_cnt = {}                        # lane name -> dma count
        self.seen = {}                            # (consumer, producer) -> val
        self.buf = {}                             # name -> dict(w=token, r=[tokens])

    def _tok_wait(self, waits, consumer, tok):
        if tok is None:
            return
        prod, val = tok
        if prod == consumer:
            return
        if isinstance(prod, tuple):
            # DMA-lane completion sems can interleave across in-flight DMAs;
            # a wait is only sound for the lane's full issued-so-far count
            # (callers guarantee no later DMA on the lane is issued before
            # this wait passes at runtime).
            val = 16 * self.lane_cnt[prod[1]]
        key = (consumer, prod)
        if self.seen.get(key, 0) >= val:
            return
        self.seen[key] = val
        waits[prod] = max(waits.get(prod, 0), val)

    def op(self, engine, emit_fn, reads=(), writes=(), lane=None):
        waits = {}
        for nm in reads:
            b = self.buf.get(nm)
            if b:
                self._tok_wait(waits, engine, b["w"])
        for nm in writes:
            b = self.buf.get(nm)
            if b:
                self._tok_wait(waits, engine, b["w"])      # WAW
                for t in b["r"]:                           # WAR
                    self._tok_wait(waits, engine, t)
        if engine in ("V", "A"):
            self.tick[engine] += 1
            tok = (engine, self.tick[engine])
        else:
            assert lane is not None
            self.lane_cnt[lane] = self.lane_cnt.get(lane, 0) + 1
            tok = (("L", lane), 16 * self.lane_cnt[lane])
        for nm in writes:
            self.buf[nm] = {"w": tok, "r": []}
        for nm in reads:
            b = self.buf.setdefault(nm, {"w": None, "r": []})
            b["r"].append(tok)
        self.ops[engine].append((waits, emit_fn, tok, lane))


def _build():
    """Build the Bass program once.  Returns (nc, None)."""
    global _BUILT
    if _BUILT is not None:
        return _BUILT
    from contextlib import ExitStack
    import concourse.bass as bass
    from concourse import mybir

    dt = mybir.dt
    Alu = mybir.AluOpType
    Act = mybir.ActivationFunctionType

    nc = bass.Bass("TRN2", target_bir_lowering=False, debug=False,
                   num_devices=NDEV)

    # ---------------- DRAM I/O ----------------
    pix_d = nc.dram_tensor("pix", [P, 10, F], dt.float32, kind="ExternalInput")
    sc_d = nc.dram_tensor("sc", [P, 8, SLOC], dt.float32, kind="ExternalInput")
    g_d = nc.dram_tensor("g", [NCH, P, SLOC, F], dt.bfloat16,
                         kind="ExternalInput")
    out_d = nc.dram_tensor("out", [NCH, P, F], dt.float32,
                           kind="ExternalOutput")
    import os
    dbg = bool(int(os.environ.get("KDEBUG", "0")))
    if dbg:
        wdbg_d = nc.dram_tensor("wdbg", [P, 3, SLOC, F], dt.bfloat16,
                                kind="ExternalOutput")
        cdbg_d = nc.dram_tensor("cdbg", [8, P, F], dt.float32,
                                kind="ExternalOutput")

    ctx = ExitStack()
    sb = lambda name, shape, dtype: ctx.enter_context(
        nc.sbuf_tensor(name, shape, dtype))

    PIX = sb("sPIX", [P, 10, F], dt.float32)
    SC = sb("sSC", [P, 8, SLOC], dt.float32)
    FLD = sb("sFLD", [P, 12, F], dt.float32)
    T = sb("sT", [P, 10, F], dt.float32)      # scratch t0..t9
    C = sb("sC", [P, 9, F], dt.float32)       # chain scratch c0..c8
    W = sb("sW", [P, 3, SLOC, F], dt.bfloat16)
    GT = sb("sGT", [P, 2, SLOC, F], dt.bfloat16)
    PR = sb("sPR", [P, SLOC, F], dt.bfloat16)
    OUTS = sb("sOUTS", [P, NCH, F], dt.float32)

    sem_v = ctx.enter_context(nc.semaphore("sem_v"))
    sem_a = ctx.enter_context(nc.semaphore("sem_a"))
    lane_sems = {}
    for ln in ["in"] + [f"g{j}" for j in range(NCH)] + ["out"]:
        lane_sems[ln] = ctx.enter_context(nc.semaphore(f"lane_{ln}"))

    pr = Prog()
    V, A = nc.vector, nc.scalar

    # helper closures -------------------------------------------------------
    def vop(emit, reads, writes):
        pr.op("V", emit, reads, writes)

    def aop(emit, reads, writes):
        pr.op("A", emit, reads, writes)

    def sop(emit, reads, writes, lane):
        pr.op("S", emit, reads, writes, lane=lane)

    # AP shorthands
    def pix(i):
        return PIX[:, i, :]
    def fld(i):
        return FLD[:, i, :]
    def t(i):
        return T[:, i, :]
    def c(i):
        return C[:, i, :]

    VXi, VYi, VZi, NXi, NYi, NZi, DF0i, DF1i, DF2i, RGi = range(10)
    # field slots
    iVCX, iVCY, iVCZ, iCXX, iDB0, iDB1, iDB2, iA2, iA2M1, iOMK, iKK, iPC = range(12)

    # ---------------- Phase 0: loads ----------------
    sop(lambda: nc.sync.dma_start(PIX[:], pix_d[:]).then_inc(lane_sems["in"], 16),
        [], ["PIX"], lane="in")
    sop(lambda: nc.sync.dma_start(SC[:], sc_d[:]).then_inc(lane_sems["in"], 16),
        [], ["SC"], lane="in")
    for j in range(2):   # prefetch first two G tiles
        def em(j=j):
            return nc.sync.dma_start(GT[:, j % 2, :, :], g_d[j, :, :, :]) \
                .then_inc(lane_sems[f"g{j}"], 16)
        sop(em, [], [f"GT{j % 2}"], lane=f"g{j}")

    # ---------------- Phase 1: per-pixel precompute ----------------
    # cy = up - (up.n) n ; up=(0,1,0)
    vop(lambda: V.scalar_tensor_tensor(t(0), pix(NXi), -1.0, pix(NYi),
                                       Alu.mult, Alu.mult),
        ["PIX"], ["t0"])                                            # cyx
    vop(lambda: V.scalar_tensor_tensor(t(1), pix(NZi), -1.0, pix(NYi),
                                       Alu.mult, Alu.mult),
        ["PIX"], ["t1"])                                            # cyz
    vop(lambda: V.scalar_tensor_tensor(t(2), pix(NYi), -1.0, pix(NYi),
                                       Alu.mult, Alu.mult),
        ["PIX"], ["t2"])                                            # -ny^2
    aop(lambda: A.activation(t(2), t(2), Act.Identity, bias=1.0),
        ["t2"], ["t2"])                                             # cyy
    aop(lambda: A.activation(t(3), t(0), Act.Square), ["t0"], ["t3"])
    aop(lambda: A.activation(t(6), t(1), Act.Square), ["t1"], ["t6"])
    vop(lambda: V.tensor_mul(t(4), t(2), t(2)), ["t2"], ["t4"])
    vop(lambda: V.tensor_add(t(5), t(3), t(4)), ["t3", "t4"], ["t5"])
    vop(lambda: V.tensor_add(t(7), t(5), t(6)), ["t5", "t6"], ["t7"])  # cc
    vop(lambda: V.tensor_scalar(t(8), t(7), 1e-24, None, Alu.max),
        ["t7"], ["t8"])
    aop(lambda: A.activation(t(9), t(8), Act.Ln), ["t8"], ["t9"])
    aop(lambda: A.activation(t(9), t(9), Act.Exp, scale=-0.5),
        ["t9"], ["t9"])  # icn ~= rsqrt(max(cc,1e-24))
    # Newton step for rsqrt accuracy (PWP-table error would otherwise leak
    # into vdl/uu and get amplified by the nom0 cancellation downstream)
    vop(lambda: V.tensor_mul(t(3), t(9), t(9)), ["t9"], ["t3"])
    vop(lambda: V.tensor_mul(t(3), t(3), t(8)), ["t3", "t8"], ["t3"])
    vop(lambda: V.tensor_scalar(t(3), t(3), -0.5, 1.5, Alu.mult, Alu.add),
        ["t3"], ["t3"])
    vop(lambda: V.tensor_mul(t(9), t(9), t(3)), ["t9", "t3"], ["t9"])
    # camy = cy * icn  -> t3,t4,t5
    vop(lambda: V.tensor_mul(t(3), t(0), t(9)), ["t0", "t9"], ["t3"])
    vop(lambda: V.tensor_mul(t(4), t(2), t(9)), ["t2", "t9"], ["t4"])
    vop(lambda: V.tensor_mul(t(5), t(1), t(9)), ["t1", "t9"], ["t5"])
    # vcy = v . camy
    vop(lambda: V.tensor_mul(t(6), pix(VXi), t(3)), ["PIX", "t3"], ["t6"])
    vop(lambda: V.tensor_mul(t(7), pix(VYi), t(4)), ["PIX", "t4"], ["t7"])
    vop(lambda: V.tensor_add(t(6), t(6), t(7)), ["t6", "t7"], ["t6"])
    vop(lambda: V.tensor_mul(t(7), pix(VZi), t(5)), ["PIX", "t5"], ["t7"])
    vop(lambda: V.tensor_add(fld(iVCY), t(6), t(7)), ["t6", "t7"], ["FVCY"])
    # d1 = |nz| + |nx| ; id1
    aop(lambda: A.activation(t(3), pix(NZi), Act.Abs), ["PIX"], ["t3"])
    aop(lambda: A.activation(t(4), pix(NXi), Act.Abs), ["PIX"], ["t4"])
    vop(lambda: V.tensor_add(t(5), t(3), t(4)), ["t3", "t4"], ["t5"])
    vop(lambda: V.tensor_scalar(t(5), t(5), 1e-30, None, Alu.max),
        ["t5"], ["t5"])
    aop(lambda: A.activation(t(8), t(5), Act.Ln), ["t5"], ["t8"])
    aop(lambda: A.activation(t(8), t(8), Act.Exp, scale=-1.0),
        ["t8"], ["t8"])  # id1 ~= 1/(|nz|+|nx|)
    # Newton step r' = r*(2 - x*r)
    vop(lambda: V.tensor_mul(t(9), t(8), t(5)), ["t8", "t5"], ["t9"])
    vop(lambda: V.tensor_scalar(t(9), t(9), -1.0, 2.0, Alu.mult, Alu.add),
        ["t9"], ["t9"])
    vop(lambda: V.tensor_mul(t(8), t(8), t(9)), ["t8", "t9"], ["t8"])
    # vcx = (-vx*nz + vz*nx) * id1
    vop(lambda: V.scalar_tensor_tensor(t(6), pix(VXi), -1.0, pix(NZi),
                                       Alu.mult, Alu.mult),
        ["PIX"], ["t6"])
    vop(lambda: V.tensor_mul(t(7), pix(VZi), pix(NXi)), ["PIX"], ["t7"])
    vop(lambda: V.tensor_add(t(6), t(6), t(7)), ["t6", "t7"], ["t6"])
    vop(lambda: V.tensor_mul(fld(iVCX), t(6), t(8)), ["t6", "t8"], ["FVCX"])
    # cxx = (nz^2 + nx^2) * id1^2
    aop(lambda: A.activation(t(6), pix(NZi), Act.Square), ["PIX"], ["t6"])
    aop(lambda: A.activation(t(7), pix(NXi), Act.Square), ["PIX"], ["t7"])
    vop(lambda: V.tensor_add(t(6), t(6), t(7)), ["t6", "t7"], ["t6"])
    vop(lambda: V.tensor_mul(t(7), t(8), t(8)), ["t8"], ["t7"])
    vop(lambda: V.tensor_mul(fld(iCXX), t(6), t(7)), ["t6", "t7"], ["FCXX"])
    # vcz = v . n
    vop(lambda: V.tensor_mul(t(6), pix(VXi), pix(NXi)), ["PIX"], ["t6"])
    vop(lambda: V.tensor_mul(t(7), pix(VYi), pix(NYi)), ["PIX"], ["t7"])
    vop(lambda: V.tensor_add(t(6), t(6), t(7)), ["t6", "t7"], ["t6"])
    vop(lambda: V.tensor_mul(t(7), pix(VZi), pix(NZi)), ["PIX"], ["t7"])
    vop(lambda: V.tensor_add(fld(iVCZ), t(6), t(7)), ["t6", "t7"], ["FVCZ"])
    # diffB_c = (df + 1) / (2*pi)
    for cch in range(3):
        def em(cch=cch):
            return V.tensor_scalar(fld(iDB0 + cch), pix(DF0i + cch),
                                   1.0 / (2 * PI), 1.0 / (2 * PI),
                                   Alu.mult, Alu.add)
        vop(em, ["PIX"], [f"FDB{cch}"])
    # rough path
    vop(lambda: V.tensor_scalar(t(6), pix(RGi), 0.5, 0.5, Alu.mult, Alu.add),
        ["PIX"], ["t6"])                                            # roughB
    aop(lambda: A.activation(t(7), t(6), Act.Square, bias=1.0), ["t6"], ["t7"])
    vop(lambda: V.tensor_scalar(fld(iKK), t(7), 0.125, None, Alu.mult),
        ["t7"], ["FKK"])
    vop(lambda: V.tensor_scalar(fld(iOMK), t(7), -0.125, 1.0, Alu.mult, Alu.add),
        ["t7"], ["FOMK"])
    aop(lambda: A.activation(t(8), t(6), Act.Square), ["t6"], ["t8"])
    aop(lambda: A.activation(fld(iA2), t(8), Act.Square), ["t8"], ["FA2"])
    vop(lambda: V.tensor_scalar(fld(iA2M1), fld(iA2), -1.0, None, Alu.add),
        ["FA2"], ["FA2M1"])
    # pc = 4*pi*(ndv*(1-k)+k),  ndv = clip(vcz,0,1)
    vop(lambda: V.tensor_scalar(t(6), fld(iVCZ), 0.0, 1.0, Alu.max, Alu.min),
        ["FVCZ"], ["t6"])
    vop(lambda: V.tensor_mul(t(7), t(6), fld(iOMK)), ["t6", "FOMK"], ["t7"])
    vop(lambda: V.tensor_add(t(7), t(7), fld(iKK)), ["t7", "FKK"], ["t7"])
    vop(lambda: V.tensor_scalar(fld(iPC), t(7), 4 * PI, None, Alu.mult),
        ["t7"], ["FPC"])

    # ---------------- Phase 2: per-sample chain ----------------
    LN2 = float(np.log(2.0))
    for s in range(SLOC):
        X = SC[:, 0, s:s + 1]
        Y = SC[:, 1, s:s + 1]
        Z = SC[:, 2, s:s + 1]
        X2 = SC[:, 3, s:s + 1]
        CCS = SC[:, 4, s:s + 1]
        AL = SC[:, 5, s:s + 1]
        BE = SC[:, 6, s:s + 1]

        vop(lambda Z=Z: V.tensor_scalar(c(0), fld(iVCZ), Z, None, Alu.mult),
            ["FVCZ", "SC"], ["c0"])
        vop(lambda Y=Y: V.scalar_tensor_tensor(c(1), fld(iVCY), Y, c(0),
                                               Alu.mult, Alu.add),
            ["FVCY", "c0", "SC"], ["c1"])
        vop(lambda X=X: V.scalar_tensor_tensor(c(0), fld(iVCX), X, c(1),
                                               Alu.mult, Alu.add),
            ["FVCX", "c1", "SC"], ["c0"])                    # vdl
        vop(lambda CCS=CCS: V.tensor_scalar(c(1), c(0), 2.0, CCS,
                                            Alu.mult, Alu.add),
            ["c0", "SC"], ["c1"])                            # 2vdl+1+lsy2+lsz2
        vop(lambda X2=X2: V.scalar_tensor_tensor(c(1), fld(iCXX), X2, c(1),
                                                 Alu.mult, Alu.add),
            ["FCXX", "c1", "SC"], ["c1"])                    # uu
        vop(lambda: V.tensor_scalar(c(1), c(1), 4e-6, None, Alu.max),
            ["c1"], ["c1"])
        aop(lambda: A.activation(c(2), c(1), Act.Ln, scale=0.25),
            ["c1"], ["c2"])
        aop(lambda: A.activation(c(3), c(2), Act.Exp, scale=-0.5),
            ["c2"], ["c3"])  # denom ~= rsqrt(clip(0.25*uu, 1e-6))
        # One Newton step d' = d*(1.5 - 0.125*m*d^2): the PWP-table error of
        # Ln/Exp (~1e-5) is amplified ~1000x by the nom0 cancellation on
        # specular-highlight pixels; refine to fp32 accuracy.
        vop(lambda: V.tensor_mul(c(2), c(3), c(3)), ["c3"], ["c2"])
        vop(lambda: V.tensor_mul(c(2), c(2), c(1)), ["c2", "c1"], ["c2"])
        vop(lambda: V.tensor_scalar(c(2), c(2), -0.125, 1.5, Alu.mult, Alu.add),
            ["c2"], ["c2"])
        vop(lambda: V.tensor_mul(c(3), c(3), c(2)), ["c3", "c2"], ["c3"])
        if dbg and s == 4:
            sop(lambda: nc.sync.dma_start(cdbg_d[0], c(3)).then_inc(
                lane_sems["out"], 16), ["c3"], [], lane="out")
        vop(lambda: V.tensor_scalar(c(4), c(0), 1.0, None, Alu.add),
            ["c0"], ["c4"])                                  # vdu
        vop(lambda: V.tensor_mul(c(4), c(4), c(3)), ["c4", "c3"], ["c4"])  # p1
        vop(lambda: V.tensor_scalar(c(5), c(4), -1.38868, -3.49158,
                                    Alu.mult, Alu.add),
            ["c4"], ["c5"])
        vop(lambda: V.tensor_mul(c(5), c(5), c(4)), ["c5", "c4"], ["c5"])  # q
        aop(lambda: A.activation(c(5), c(5), Act.Exp, scale=LN2),
            ["c5"], ["c5"])
        vop(lambda: V.tensor_scalar(c(5), c(5), 1.0 - F0C, F0C,
                                    Alu.mult, Alu.add),
            ["c5"], ["c5"])                                  # e1
        if dbg and s == 4:
            sop(lambda: nc.sync.dma_start(cdbg_d[6], c(5)).then_inc(
                lane_sems["out"], 16), ["c5"], [], lane="out")
        vop(lambda Z=Z: V.tensor_scalar(c(6), fld(iVCZ), Z, None, Alu.add),
            ["FVCZ", "SC"], ["c6"])                          # ndu
        vop(lambda: V.tensor_mul(c(6), c(6), c(3)), ["c6", "c3"], ["c6"])
        vop(lambda: V.tensor_scalar(c(6), c(6), 0.5, 1.0, Alu.mult, Alu.min),
            ["c6"], ["c6"])
        vop(lambda: V.tensor_scalar(c(6), c(6), 0.0, None, Alu.max),
            ["c6"], ["c6"])                                  # ndh
        if dbg and s == 4:
            sop(lambda: nc.sync.dma_start(cdbg_d[1], c(6)).then_inc(
                lane_sems["out"], 16), ["c6"], [], lane="out")
        aop(lambda: A.activation(c(7), c(6), Act.Square), ["c6"], ["c7"])
        vop(lambda: V.tensor_mul(c(7), c(7), fld(iA2M1)),
            ["c7", "FA2M1"], ["c7"])
        if dbg and s == 4:
            sop(lambda: nc.sync.dma_start(cdbg_d[2], c(7)).then_inc(
                lane_sems["out"], 16), ["c7"], [], lane="out")
        aop(lambda: A.activation(c(7), c(7), Act.Square, bias=1.0),
            ["c7"], ["c7"])                                  # nom0^2
        if dbg and s == 4:
            sop(lambda: nc.sync.dma_start(cdbg_d[3], c(7)).then_inc(
                lane_sems["out"], 16), ["c7"], [], lane="out")
        vop(lambda Z=Z: V.scalar_tensor_tensor(c(8), fld(iOMK), Z, fld(iKK),
                                               Alu.mult, Alu.add),
            ["FOMK", "FKK", "SC"], ["c8"])                   # nom2
        vop(lambda: V.tensor_mul(c(7), c(7), fld(iPC)), ["c7", "FPC"], ["c7"])
        vop(lambda: V.tensor_mul(c(7), c(7), c(8)), ["c7", "c8"], ["c7"])
        vop(lambda: V.tensor_scalar(c(7), c(7), 1e-6, None, Alu.max),
            ["c7"], ["c7"])
        if dbg and s == 4:
            sop(lambda: nc.sync.dma_start(cdbg_d[4], c(7)).then_inc(
                lane_sems["out"], 16), ["c7"], [], lane="out")
        aop(lambda: A.activation(c(7), c(7), Act.Ln), ["c7"], ["c7"])
        aop(lambda: A.activation(c(8), c(7), Act.Exp, scale=-1.0),
            ["c7"], ["c8"])  # rnom = 1/nom
        if dbg and s == 4:
            sop(lambda: nc.sync.dma_start(cdbg_d[5], c(8)).then_inc(
                lane_sems["out"], 16), ["c8"], [], lane="out")
        vop(lambda: V.tensor_mul(c(8), c(8), c(5)), ["c8", "c5"], ["c8"])
        vop(lambda BE=BE: V.scalar_tensor_tensor(c(8), fld(iA2), BE, c(8),
                                                 Alu.mult, Alu.mult),
            ["FA2", "c8", "SC"], ["c8"])                     # sw
        if dbg and s == 4:
            sop(lambda: nc.sync.dma_start(cdbg_d[7], c(8)).then_inc(
                lane_sems["out"], 16), ["c8"], [], lane="out")
        for cch in range(3):
            def em(cch=cch, s=s, AL=AL):
                return V.scalar_tensor_tensor(W[:, cch, s, :], fld(iDB0 + cch),
                                              AL, c(8), Alu.mult, Alu.add)
            vop(em, [f"FDB{cch}", "c8", "SC"], [f"W{cch}_{s}"])

    if dbg:
        wr_all = [f"W{cc_}_{s_}" for cc_ in range(3) for s_ in range(SLOC)]
        sop(lambda: nc.sync.dma_start(wdbg_d[:], W[:]).then_inc(
            lane_sems["out"], 16), wr_all, [], lane="out")

    # ---------------- Phase 3: MAC over samples per (env,channel) ----------
    for j in range(NCH):
        b = j % 2
        cch = j % 3
        wr = [f"W{cch}_{s}" for s in range(SLOC)]
        vop(lambda j=j, b=b, cch=cch: V.tensor_mul(PR[:], W[:, cch, :, :],
                                                   GT[:, b, :, :]),
            wr + [f"GT{b}"], ["PR"])
        if j + 2 < NCH:
            def em(j=j, b=b):
                return nc.sync.dma_start(GT[:, b, :, :], g_d[j + 2, :, :, :]) \
                    .then_inc(lane_sems[f"g{j + 2}"], 16)
            sop(em, [], [f"GT{b}"], lane=f"g{j + 2}")
        h = SLOC // 2
        while h >= 1:
            if h > 1:
                def em(h=h):
                    return V.tensor_add(PR[:, 0:h, :], PR[:, 0:h, :],
                                        PR[:, h:2 * h, :])
                vop(em, ["PR"], ["PR"])
            else:
                def em(j=j):
                    return V.tensor_add(OUTS[:, j, :], PR[:, 0, :], PR[:, 1, :])
                vop(em, ["PR"], [f"OUTS{j}"])
            h //= 2
        def em(j=j):
            return nc.sync.dma_start(out_d[j, :, :], OUTS[:, j, :]) \
                .then_inc(lane_sems["out"], 16)
        sop(em, [f"OUTS{j}"], [], lane="out")

    # ---------------- Emit ----------------
    sems = {"V": sem_v, "A": sem_a}

    def emit_stream(key, eng):
        for waits, emit_fn, tok, lane in pr.ops[key]:
            for prod, val in waits.items():
                if isinstance(prod, tuple):        # ('L', lane)
                    eng.wait_ge(lane_sems[prod[1]], val)
                else:
                    eng.wait_ge(sems[prod], val)
            ins = emit_fn()
            if key in ("V", "A"):
                ins.then_inc(sems[key], 1)

    with nc.Block() as block:
        @block.sync
        def _(eng):
            emit_stream("S", nc.sync)

        @block.vector
        def _(eng):
            emit_stream("V", nc.vector)

        @block.scalar
        def _(eng):
            emit_stream("A", nc.scalar)

    ctx.close()
    _BUILT = (nc, None)
    return _BUILT


# ----------------------------------------------------------------------------
# Host side
# ----------------------------------------------------------------------------
def _host_prep(x, y, diffuse, normal, rough, v, ls, envWeight, idy, idx):
    """Returns (pix_host, sc_per_dev, g_per_dev)."""
    # per-pixel input slab [P, 10, F]
    fields = [v[0], v[1], v[2], normal[0], normal[1], normal[2],
              diffuse[0], diffuse[1], diffuse[2], rough[0]]
    pixh = np.stack([f.reshape(P, F) for f in
                     [np.asarray(a, np.float32).reshape(NPIX) for a in fields]],
                    axis=1)  # [P, 10, F]
    pixh = np.ascontiguousarray(pixh, dtype=np.float32)

    lsx = np.asarray(ls[0, :, 0], np.float64)
    lsy = np.asarray(ls[0, :, 1], np.float64)
    lsz = np.asarray(ls[0, :, 2], np.float64)
    ew = np.asarray(envWeight[0, :, 0], np.float64)
    ndl = np.clip(lsz, 0.0, 1.0)
    consts = np.zeros((8, S), np.float32)
    consts[0] = lsx
    consts[1] = lsy
    consts[2] = lsz
    consts[3] = lsx ** 2
    consts[4] = 1.0 + lsy ** 2 + lsz ** 2
    consts[5] = ew * ndl                # alpha
    consts[6] = 10.0 * ew * ndl         # beta

    # gathered envmaps: envs [4, EXH*EXW, 3]
    envs = np.concatenate([np.asarray(x, np.float32).reshape(2, EXH * EXW, 3),
                           np.asarray(y, np.float32).reshape(2, EXH * EXW, 3)],
                          axis=0)
    t = (np.asarray(idy, np.int64) * EXW + np.asarray(idx, np.int64)) \
        .reshape(S, NPIX)

    sc_per_dev, g_per_dev = [], []
    for d in range(NDEV):
        sl = slice(SLOC * d, SLOC * (d + 1))
        scd = np.ascontiguousarray(
            np.broadcast_to(consts[:, sl][None, :, :], (P, 8, SLOC)),
            dtype=np.float32)
        td = t[sl]                                   # [SLOC, NPIX]
        g = envs[:, td, :]                           # [4, SLOC, NPIX, 3]
        g = np.transpose(g, (0, 3, 1, 2)).reshape(NCH, SLOC, P, F)
        g = np.transpose(g, (0, 2, 1, 3))            # [NCH, P, SLOC, F]
        g_per_dev.append(np.ascontiguousarray(g).astype(ml_dtypes.bfloat16))
        sc_per_dev.append(scd)
    return pixh, sc_per_dev, g_per_dev


def kernel(x, y, diffuse, normal, rough, seg, v, ls, envWeight, idy, idx):
    from concourse.bass_utils import run_bass_kernel_spmd

    nc, _ = _build()
    pixh, sc_per_dev, g_per_dev = _host_prep(
        x, y, diffuse, normal, rough, v, ls, envWeight, idy, idx)

    in_maps = [{"pix": pixh, "sc": sc_per_dev[d], "g": g_per_dev[d]}
               for d in range(NDEV)]
    res = run_bass_kernel_spmd(nc, in_maps, core_ids=list(range(NDEV)),
                               trace=False)

    total = np.zeros((NCH, P, F), np.float64)
    for d in range(NDEV):
        total += res.results[d]["out"].astype(np.float64)
    fields = total.reshape(4, 3, IM, IM).astype(np.float32)
    pred = fields[0:2]          # envs from x
    gt = fields[2:4]            # envs from y
    pixel_num = float(np.asarray(seg, np.float64).sum()) * pred.shape[0] * 3
    diff = pred.astype(np.float64) - gt.astype(np.float64)
    loss = np.float32((diff ** 2).sum() / pixel_num)
    return (loss, pred[0], gt[0])


# revision 13
# speedup vs baseline: 1.0314x; 1.0314x over previous
"""EnvMap BRDF render-loss kernel for 8 Trainium2 NeuronCores.

Strategy (matches the sharding hint): shard the S=128 sample axis across the
8 cores (16 samples each).  The envmap gather (env[:, idy, idx, :] for the 4
environment images x0,x1,y0,y1) is performed host-side and the gathered
tensors are sharded along S; each core streams its [12 env-channels, 16
samples, 65536 pixels] bf16 slab and computes the full per-(sample,pixel)
BRDF chain + weighted accumulation, producing partial sums [12, 65536] f32.
Host reduces partials over cores, forms pred/gt and the scalar loss.

All math is algebraically identical to the reference:
  camy = normalize(up - (up.n)n);  cross(camy,n) = cross(up,n)/|cy| =
  (nz,0,-nx)/|cy|  =>  camx = (-nz,0,nx)/(|nz|+|nx|)  (L1-normalized).
  n.l = ls_z (basis orthogonality), v.v = 1, |camy| = 1.
  out_{e,c}[pix] = sum_s (alpha_s*diffB_c + beta_s*spec_{s,pix}) * G_{e,c,s,pix}
  with alpha_s = ew_s*ls_z_s, beta_s = 10*ew_s*ls_z_s.
"""

import numpy as np
import ml_dtypes

IM = 256
NPIX = IM * IM          # 65536
EXH, EXW = 128, 256
S = 128
NDEV = 8
SLOC = S // NDEV        # 16 samples per device
P = 128                 # partitions
F = NPIX // P           # 512 free-dim pixels
F0C = 0.05
NCH = 12                # 4 envs x 3 channels
PI = float(np.pi)

_BUILT = None           # cached (nc, meta)


# ----------------------------------------------------------------------------
# Dependency tracker: records ops per engine with explicit semaphore waits.
# ----------------------------------------------------------------------------
class Prog:
    def __init__(self):
        self.ops = {"V": [], "A": [], "S": []}   # per-engine [(waits, emit_fn)]
        self.tick = {"V": 0, "A": 0}
        self.lane_cnt = {}                        # lane name -> dma count
        self.seen = {}                            # (consumer, producer) -> val
        self.buf = {}                             # name -> dict(w=token, r=[tokens])

    def _tok_wait(self, waits, consumer, tok):
        if tok is None:
            return
        prod, val = tok
        if prod == consumer:
            return
        if isinstance(prod, tuple):
            # DMA-lane completion sems can interleave across in-flight DMAs;
            # a wait is only sound for the lane's full issued-so-far count
            # (callers guarantee no later DMA on the lane is issued before
            # this wait passes at runtime).
            val = 16 * self.lane_cnt[prod[1]]
        key = (consumer, prod)
        if self.seen.get(key, 0) >= val:
            return
        self.seen[key] = val
        waits[prod] = max(waits.get(prod, 0), val)

    def op(self, engine, emit_fn, reads=(), writes=(), lane=None):
        waits = {}
        for nm in reads:
            b = self.buf.get(nm)
            if b:
                self._tok_wait(waits, engine, b["w"])
        for nm in writes:
            b = self.buf.get(nm)
            if b:
                self._tok_wait(waits, engine, b["w"])      # WAW
                for t in b["r"]:                           # WAR
                    self._tok_wait(waits, engine, t)
        if engine in ("V", "A"):
            self.tick[engine] += 1
            tok = (engine, self.tick[engine])
        else:
            assert lane is not None
            self.lane_cnt[lane] = self.lane_cnt.get(lane, 0) + 1
            tok = (("L", lane), 16 * self.lane_cnt[lane])
        for nm in writes:
            self.buf[nm] = {"w": tok, "r": []}
        for nm in reads:
            b = self.buf.setdefault(nm, {"w": None, "r": []})
            b["r"].append(tok)
        self.ops[engine].append((waits, emit_fn, tok, lane))


def _build():
    """Build the Bass program once.  Returns (nc, None)."""
    global _BUILT
    if _BUILT is not None:
        return _BUILT
    from contextlib import ExitStack
    import concourse.bass as bass
    from concourse import mybir

    dt = mybir.dt
    Alu = mybir.AluOpType
    Act = mybir.ActivationFunctionType

    nc = bass.Bass("TRN2", target_bir_lowering=False, debug=False,
                   num_devices=NDEV)

    # ---------------- DRAM I/O ----------------
    pix_d = nc.dram_tensor("pix", [P, 10, F], dt.float32, kind="ExternalInput")
    sc_d = nc.dram_tensor("sc", [P, 8, SLOC], dt.float32, kind="ExternalInput")
    g_d = nc.dram_tensor("g", [NCH, P, SLOC, F], dt.bfloat16,
                         kind="ExternalInput")
    out_d = nc.dram_tensor("out", [NCH, P, F], dt.float32,
                           kind="ExternalOutput")
    import os
    dbg = bool(int(os.environ.get("KDEBUG", "0")))
    if dbg:
        wdbg_d = nc.dram_tensor("wdbg", [P, 3, SLOC, F], dt.bfloat16,
                                kind="ExternalOutput")
        cdbg_d = nc.dram_tensor("cdbg", [8, P, F], dt.float32,
                                kind="ExternalOutput")

    ctx = ExitStack()
    sb = lambda name, shape, dtype: ctx.enter_context(
        nc.sbuf_tensor(name, shape, dtype))

    PIX = sb("sPIX", [P, 10, F], dt.float32)
    SC = sb("sSC", [P, 8, SLOC], dt.float32)
    FLD = sb("sFLD", [P, 12, F], dt.float32)
    T = sb("sT", [P, 10, F], dt.float32)      # scratch t0..t9
    C = sb("sC", [P, 9, F], dt.float32)       # chain scratch c0..c8
    W = sb("sW", [P, 3, SLOC, F], dt.bfloat16)
    GT = sb("sGT", [P, 2, SLOC, F], dt.bfloat16)
    PR = sb("sPR", [P, SLOC, F], dt.bfloat16)
    OUTS = sb("sOUTS", [P, NCH, F], dt.float32)
    CB = sb("sCB", [P, 2], dt.float32)        # bias constants

    sem_v = ctx.enter_context(nc.semaphore("sem_v"))
    sem_a = ctx.enter_context(nc.semaphore("sem_a"))
    lane_sems = {}
    for ln in ["in"] + [f"g{j}" for j in range(NCH)] + ["out"]:
        lane_sems[ln] = ctx.enter_context(nc.semaphore(f"lane_{ln}"))

    pr = Prog()
    V, A = nc.vector, nc.scalar

    # helper closures -------------------------------------------------------
    def vop(emit, reads, writes):
        pr.op("V", emit, reads, writes)

    def aop(emit, reads, writes):
        pr.op("A", emit, reads, writes)

    def sop(emit, reads, writes, lane):
        pr.op("S", emit, reads, writes, lane=lane)

    # AP shorthands
    def pix(i):
        return PIX[:, i, :]
    def fld(i):
        return FLD[:, i, :]
    def t(i):
        return T[:, i, :]
    def c(i):
        return C[:, i, :]

    VXi, VYi, VZi, NXi, NYi, NZi, DF0i, DF1i, DF2i, RGi = range(10)
    # field slots
    iVCX, iVCY, iVCZ, iCXX, iDB0, iDB1, iDB2, iA2, iA2M1, iOMK, iKK, iPC = range(12)

    # ---------------- Phase 0: loads ----------------
    sop(lambda: nc.sync.dma_start(PIX[:], pix_d[:]).then_inc(lane_sems["in"], 16),
        [], ["PIX"], lane="in")
    sop(lambda: nc.sync.dma_start(SC[:], sc_d[:]).then_inc(lane_sems["in"], 16),
        [], ["SC"], lane="in")
    for j in range(2):   # prefetch first two G tiles
        def em(j=j):
            return nc.sync.dma_start(GT[:, j % 2, :, :], g_d[j, :, :, :]) \
                .then_inc(lane_sems[f"g{j}"], 16)
        sop(em, [], [f"GT{j % 2}"], lane=f"g{j}")

    # ---------------- Phase 1: per-pixel precompute ----------------
    vop(lambda: V.memset(CB[:, 0:1], -3.49158), [], ["CB"])
    vop(lambda: V.memset(CB[:, 1:2], 0.05), ["CB"], ["CB"])
    # cy = up - (up.n) n ; up=(0,1,0)
    vop(lambda: V.scalar_tensor_tensor(t(0), pix(NXi), -1.0, pix(NYi),
                                       Alu.mult, Alu.mult),
        ["PIX"], ["t0"])                                            # cyx
    vop(lambda: V.scalar_tensor_tensor(t(1), pix(NZi), -1.0, pix(NYi),
                                       Alu.mult, Alu.mult),
        ["PIX"], ["t1"])                                            # cyz
    vop(lambda: V.scalar_tensor_tensor(t(2), pix(NYi), -1.0, pix(NYi),
                                       Alu.mult, Alu.mult),
        ["PIX"], ["t2"])                                            # -ny^2
    aop(lambda: A.activation(t(2), t(2), Act.Identity, bias=1.0),
        ["t2"], ["t2"])                                             # cyy
    aop(lambda: A.activation(t(3), t(0), Act.Square), ["t0"], ["t3"])
    aop(lambda: A.activation(t(6), t(1), Act.Square), ["t1"], ["t6"])
    vop(lambda: V.tensor_mul(t(4), t(2), t(2)), ["t2"], ["t4"])
    vop(lambda: V.tensor_add(t(5), t(3), t(4)), ["t3", "t4"], ["t5"])
    vop(lambda: V.tensor_add(t(7), t(5), t(6)), ["t5", "t6"], ["t7"])  # cc
    vop(lambda: V.tensor_scalar(t(8), t(7), 1e-24, None, Alu.max),
        ["t7"], ["t8"])
    aop(lambda: A.activation(t(9), t(8), Act.Ln), ["t8"], ["t9"])
    aop(lambda: A.activation(t(9), t(9), Act.Exp, scale=-0.5),
        ["t9"], ["t9"])  # icn ~= rsqrt(max(cc,1e-24))
    # Newton step for rsqrt accuracy (PWP-table error would otherwise leak
    # into vdl/uu and get amplified by the nom0 cancellation downstream)
    vop(lambda: V.tensor_mul(t(3), t(9), t(9)), ["t9"], ["t3"])
    vop(lambda: V.tensor_mul(t(3), t(3), t(8)), ["t3", "t8"], ["t3"])
    vop(lambda: V.tensor_scalar(t(3), t(3), -0.5, 1.5, Alu.mult, Alu.add),
        ["t3"], ["t3"])
    vop(lambda: V.tensor_mul(t(9), t(9), t(3)), ["t9", "t3"], ["t9"])
    # camy = cy * icn  -> t3,t4,t5
    vop(lambda: V.tensor_mul(t(3), t(0), t(9)), ["t0", "t9"], ["t3"])
    vop(lambda: V.tensor_mul(t(4), t(2), t(9)), ["t2", "t9"], ["t4"])
    vop(lambda: V.tensor_mul(t(5), t(1), t(9)), ["t1", "t9"], ["t5"])
    # vcy = v . camy
    vop(lambda: V.tensor_mul(t(6), pix(VXi), t(3)), ["PIX", "t3"], ["t6"])
    vop(lambda: V.tensor_mul(t(7), pix(VYi), t(4)), ["PIX", "t4"], ["t7"])
    vop(lambda: V.tensor_add(t(6), t(6), t(7)), ["t6", "t7"], ["t6"])
    vop(lambda: V.tensor_mul(t(7), pix(VZi), t(5)), ["PIX", "t5"], ["t7"])
    vop(lambda: V.tensor_add(fld(iVCY), t(6), t(7)), ["t6", "t7"], ["FVCY"])
    # d1 = |nz| + |nx| ; id1
    aop(lambda: A.activation(t(3), pix(NZi), Act.Abs), ["PIX"], ["t3"])
    aop(lambda: A.activation(t(4), pix(NXi), Act.Abs), ["PIX"], ["t4"])
    vop(lambda: V.tensor_add(t(5), t(3), t(4)), ["t3", "t4"], ["t5"])
    vop(lambda: V.tensor_scalar(t(5), t(5), 1e-30, None, Alu.max),
        ["t5"], ["t5"])
    aop(lambda: A.activation(t(8), t(5), Act.Ln), ["t5"], ["t8"])
    aop(lambda: A.activation(t(8), t(8), Act.Exp, scale=-1.0),
        ["t8"], ["t8"])  # id1 ~= 1/(|nz|+|nx|)
    # Newton step r' = r*(2 - x*r)
    vop(lambda: V.tensor_mul(t(9), t(8), t(5)), ["t8", "t5"], ["t9"])
    vop(lambda: V.tensor_scalar(t(9), t(9), -1.0, 2.0, Alu.mult, Alu.add),
        ["t9"], ["t9"])
    vop(lambda: V.tensor_mul(t(8), t(8), t(9)), ["t8", "t9"], ["t8"])
    # vcx = (-vx*nz + vz*nx) * id1
    vop(lambda: V.scalar_tensor_tensor(t(6), pix(VXi), -1.0, pix(NZi),
                                       Alu.mult, Alu.mult),
        ["PIX"], ["t6"])
    vop(lambda: V.tensor_mul(t(7), pix(VZi), pix(NXi)), ["PIX"], ["t7"])
    vop(lambda: V.tensor_add(t(6), t(6), t(7)), ["t6", "t7"], ["t6"])
    vop(lambda: V.tensor_mul(fld(iVCX), t(6), t(8)), ["t6", "t8"], ["FVCX"])
    # cxx = (nz^2 + nx^2) * id1^2
    aop(lambda: A.activation(t(6), pix(NZi), Act.Square), ["PIX"], ["t6"])
    aop(lambda: A.activation(t(7), pix(NXi), Act.Square), ["PIX"], ["t7"])
    vop(lambda: V.tensor_add(t(6), t(6), t(7)), ["t6", "t7"], ["t6"])
    vop(lambda: V.tensor_mul(t(7), t(8), t(8)), ["t8"], ["t7"])
    vop(lambda: V.tensor_mul(fld(iCXX), t(6), t(7)), ["t6", "t7"], ["FCXX"])
    # vcz = v . n
    vop(lambda: V.tensor_mul(t(6), pix(VXi), pix(NXi)), ["PIX"], ["t6"])
    vop(lambda: V.tensor_mul(t(7), pix(VYi), pix(NYi)), ["PIX"], ["t7"])
    vop(lambda: V.tensor_add(t(6), t(6), t(7)), ["t6", "t7"], ["t6"])
    vop(lambda: V.tensor_mul(t(7), pix(VZi), pix(NZi)), ["PIX"], ["t7"])
    vop(lambda: V.tensor_add(fld(iVCZ), t(6), t(7)), ["t6", "t7"], ["FVCZ"])
    # diffB_c = (df + 1) / (2*pi)
    for cch in range(3):
        def em(cch=cch):
            return V.tensor_scalar(fld(iDB0 + cch), pix(DF0i + cch),
                                   1.0 / (2 * PI), 1.0 / (2 * PI),
                                   Alu.mult, Alu.add)
        vop(em, ["PIX"], [f"FDB{cch}"])
    # rough path
    vop(lambda: V.tensor_scalar(t(6), pix(RGi), 0.5, 0.5, Alu.mult, Alu.add),
        ["PIX"], ["t6"])                                            # roughB
    aop(lambda: A.activation(t(7), t(6), Act.Square, bias=1.0), ["t6"], ["t7"])
    vop(lambda: V.tensor_scalar(t(9), t(7), 0.125, None, Alu.mult),
        ["t7"], ["t9"])                                             # kk
    vop(lambda: V.tensor_scalar(t(5), t(7), -0.125, 1.0, Alu.mult, Alu.add),
        ["t7"], ["t5"])                                             # omk
    aop(lambda: A.activation(t(8), t(6), Act.Square), ["t6"], ["t8"])
    aop(lambda: A.activation(fld(iA2), t(8), Act.Square), ["t8"], ["FA2"])
    vop(lambda: V.tensor_scalar(fld(iA2M1), fld(iA2), -1.0, None, Alu.add),
        ["FA2"], ["FA2M1"])
    # pc = 4*pi*(ndv*(1-k)+k),  ndv = clip(vcz,0,1)
    vop(lambda: V.tensor_scalar(t(6), fld(iVCZ), 0.0, 1.0, Alu.max, Alu.min),
        ["FVCZ"], ["t6"])
    vop(lambda: V.tensor_mul(t(7), t(6), t(5)), ["t6", "t5"], ["t7"])
    vop(lambda: V.tensor_add(t(7), t(7), t(9)), ["t7", "t9"], ["t7"])
    vop(lambda: V.tensor_scalar(t(8), t(7), 4 * PI, None, Alu.mult),
        ["t7"], ["t8"])                                             # pc
    vop(lambda: V.tensor_mul(fld(iOMK), t(8), t(5)), ["t8", "t5"], ["FOMK"])
    vop(lambda: V.tensor_mul(fld(iKK), t(8), t(9)), ["t8", "t9"], ["FKK"])

    # ---------------- Phase 2: per-sample chain ----------------
    LN2 = float(np.log(2.0))
    for s in range(SLOC):
        X = SC[:, 0, s:s + 1]
        Y = SC[:, 1, s:s + 1]
        Z = SC[:, 2, s:s + 1]
        X2 = SC[:, 3, s:s + 1]
        CCS = SC[:, 4, s:s + 1]
        AL = SC[:, 5, s:s + 1]
        BE = SC[:, 6, s:s + 1]

        aop(lambda Z=Z: A.activation(c(0), fld(iVCZ), Act.Copy, scale=Z),
            ["FVCZ", "SC"], ["c0"])
        vop(lambda Y=Y: V.scalar_tensor_tensor(c(1), fld(iVCY), Y, c(0),
                                               Alu.mult, Alu.add),
            ["FVCY", "c0", "SC"], ["c1"])
        vop(lambda X=X: V.scalar_tensor_tensor(c(0), fld(iVCX), X, c(1),
                                               Alu.mult, Alu.add),
            ["FVCX", "c1", "SC"], ["c0"])                    # vdl
        aop(lambda CCS=CCS: A.activation(c(1), c(0), Act.Identity,
                                         scale=2.0, bias=CCS),
            ["c0", "SC"], ["c1"])                            # 2vdl+1+lsy2+lsz2
        vop(lambda X2=X2: V.scalar_tensor_tensor(c(1), fld(iCXX), X2, c(1),
                                                 Alu.mult, Alu.add),
            ["FCXX", "c1", "SC"], ["c1"])                    # uu
        vop(lambda: V.tensor_scalar(c(1), c(1), 4e-6, None, Alu.max),
            ["c1"], ["c1"])
        aop(lambda: A.activation(c(2), c(1), Act.Ln, scale=0.25),
            ["c1"], ["c2"])
        aop(lambda: A.activation(c(3), c(2), Act.Exp, scale=-0.5),
            ["c2"], ["c3"])  # denom ~= rsqrt(clip(0.25*uu, 1e-6))
        # One Newton step d' = d*(1.5 - 0.125*m*d^2): the PWP-table error of
        # Ln/Exp (~1e-5) is amplified ~1000x by the nom0 cancellation on
        # specular-highlight pixels; refine to fp32 accuracy.
        vop(lambda: V.tensor_mul(c(2), c(3), c(3)), ["c3"], ["c2"])
        vop(lambda: V.tensor_mul(c(2), c(2), c(1)), ["c2", "c1"], ["c2"])
        vop(lambda: V.tensor_scalar(c(2), c(2), -0.125, 1.5, Alu.mult, Alu.add),
            ["c2"], ["c2"])
        vop(lambda: V.tensor_mul(c(3), c(3), c(2)), ["c3", "c2"], ["c3"])
        if dbg and s == 4:
            sop(lambda: nc.sync.dma_start(cdbg_d[0], c(3)).then_inc(
                lane_sems["out"], 16), ["c3"], [], lane="out")
        vop(lambda: V.scalar_tensor_tensor(c(4), c(0), 1.0, c(3),
                                           Alu.add, Alu.mult),
            ["c0", "c3"], ["c4"])                            # p1 = (1+vdl)*denom
        aop(lambda: A.activation(c(5), c(4), Act.Identity, scale=-1.38868,
                                 bias=CB[:, 0:1]),
            ["c4", "CB"], ["c5"])
        vop(lambda: V.tensor_mul(c(5), c(5), c(4)), ["c5", "c4"], ["c5"])  # q
        aop(lambda: A.activation(c(5), c(5), Act.Exp, scale=LN2),
            ["c5"], ["c5"])
        aop(lambda: A.activation(c(5), c(5), Act.Identity, scale=1.0 - F0C,
                                 bias=CB[:, 1:2]),
            ["c5", "CB"], ["c5"])                            # e1
        if dbg and s == 4:
            sop(lambda: nc.sync.dma_start(cdbg_d[6], c(5)).then_inc(
                lane_sems["out"], 16), ["c5"], [], lane="out")
        vop(lambda Z=Z: V.scalar_tensor_tensor(c(6), fld(iVCZ), Z, c(3),
                                               Alu.add, Alu.mult),
            ["FVCZ", "c3", "SC"], ["c6"])                    # ndu*denom
        # ndh = max(0, 0.5*ndu*denom); the <=1 clip is redundant by
        # Cauchy-Schwarz (|n.u| <= |u|), up to ~1e-7 rounding which is
        # harmless since nom0 is squared.
        aop(lambda: A.activation(c(6), c(6), Act.Relu, scale=0.5),
            ["c6"], ["c6"])                                  # ndh
        if dbg and s == 4:
            sop(lambda: nc.sync.dma_start(cdbg_d[1], c(6)).then_inc(
                lane_sems["out"], 16), ["c6"], [], lane="out")
        aop(lambda: A.activation(c(7), c(6), Act.Square), ["c6"], ["c7"])
        vop(lambda: V.tensor_mul(c(7), c(7), fld(iA2M1)),
            ["c7", "FA2M1"], ["c7"])
        if dbg and s == 4:
            sop(lambda: nc.sync.dma_start(cdbg_d[2], c(7)).then_inc(
                lane_sems["out"], 16), ["c7"], [], lane="out")
        aop(lambda: A.activation(c(7), c(7), Act.Square, bias=1.0),
            ["c7"], ["c7"])                                  # nom0^2
        if dbg and s == 4:
            sop(lambda: nc.sync.dma_start(cdbg_d[3], c(7)).then_inc(
                lane_sems["out"], 16), ["c7"], [], lane="out")
        vop(lambda Z=Z: V.scalar_tensor_tensor(c(8), fld(iOMK), Z, fld(iKK),
                                               Alu.mult, Alu.add),
            ["FOMK", "FKK", "SC"], ["c8"])                   # pc*nom2
        vop(lambda: V.tensor_mul(c(7), c(7), c(8)), ["c7", "c8"], ["c7"])
        vop(lambda: V.tensor_scalar(c(7), c(7), 1e-6, None, Alu.max),
            ["c7"], ["c7"])
        if dbg and s == 4:
            sop(lambda: nc.sync.dma_start(cdbg_d[4], c(7)).then_inc(
                lane_sems["out"], 16), ["c7"], [], lane="out")
        aop(lambda: A.activation(c(7), c(7), Act.Ln), ["c7"], ["c7"])
        aop(lambda: A.activation(c(8), c(7), Act.Exp, scale=-1.0),
            ["c7"], ["c8"])  # rnom = 1/nom
        if dbg and s == 4:
            sop(lambda: nc.sync.dma_start(cdbg_d[5], c(8)).then_inc(
                lane_sems["out"], 16), ["c8"], [], lane="out")
        vop(lambda: V.tensor_mul(c(8), c(8), c(5)), ["c8", "c5"], ["c8"])
        vop(lambda BE=BE: V.scalar_tensor_tensor(c(8), fld(iA2), BE, c(8),
                                                 Alu.mult, Alu.mult),
            ["FA2", "c8", "SC"], ["c8"])                     # sw
        if dbg and s == 4:
            sop(lambda: nc.sync.dma_start(cdbg_d[7], c(8)).then_inc(
                lane_sems["out"], 16), ["c8"], [], lane="out")
        def em(s=s, AL=AL):
            return V.scalar_tensor_tensor(
                W[:, :, s, :], FLD[:, iDB0:iDB0 + 3, :], AL,
                C[:, 8:9, :].to_broadcast([P, 3, F]), Alu.mult, Alu.add)
        vop(em, ["FDB0", "FDB1", "FDB2", "c8", "SC"],
            [f"W0_{s}", f"W1_{s}", f"W2_{s}"])

    if dbg:
        wr_all = [f"W{cc_}_{s_}" for cc_ in range(3) for s_ in range(SLOC)]
        sop(lambda: nc.sync.dma_start(wdbg_d[:], W[:]).then_inc(
            lane_sems["out"], 16), wr_all, [], lane="out")

    # ---------------- Phase 3: MAC over samples per (env,channel) ----------
    for j in range(NCH):
        b = j % 2
        cch = j % 3
        wr = [f"W{cch}_{s}" for s in range(SLOC)]
        vop(lambda j=j, b=b, cch=cch: V.tensor_mul(PR[:], W[:, cch, :, :],
                                                   GT[:, b, :, :]),
            wr + [f"GT{b}"], ["PR"])
        if j + 2 < NCH:
            def em(j=j, b=b):
                return nc.sync.dma_start(GT[:, b, :, :], g_d[j + 2, :, :, :]) \
                    .then_inc(lane_sems[f"g{j + 2}"], 16)
            sop(em, [], [f"GT{b}"], lane=f"g{j + 2}")
        h = SLOC // 2
        while h >= 1:
            if h > 1:
                def em(h=h):
                    return V.tensor_add(PR[:, 0:h, :], PR[:, 0:h, :],
                                        PR[:, h:2 * h, :])
                vop(em, ["PR"], ["PR"])
            else:
                def em(j=j):
                    return V.tensor_add(OUTS[:, j, :], PR[:, 0, :], PR[:, 1, :])
                vop(em, ["PR"], [f"OUTS{j}"])
            h //= 2
        def em(j=j):
            return nc.sync.dma_start(out_d[j, :, :], OUTS[:, j, :]) \
                .then_inc(lane_sems["out"], 16)
        sop(em, [f"OUTS{j}"], [], lane="out")

    # ---------------- Emit ----------------
    sems = {"V": sem_v, "A": sem_a}

    def emit_stream(key, eng):
        for waits, emit_fn, tok, lane in pr.ops[key]:
            for prod, val in waits.items():
                if isinstance(prod, tuple):        # ('L', lane)
                    eng.wait_ge(lane_sems[prod[1]], val)
                else:
                    eng.wait_ge(sems[prod], val)
            ins = emit_fn()
            if key in ("V", "A"):
                ins.then_inc(sems[key], 1)

    with nc.Block() as block:
        @block.sync
        def _(eng):
            emit_stream("S", nc.sync)

        @block.vector
        def _(eng):
            emit_stream("V", nc.vector)

        @block.scalar
        def _(eng):
            emit_stream("A", nc.scalar)

    ctx.close()
    _BUILT = (nc, None)
    return _BUILT


# ----------------------------------------------------------------------------
# Host side
# ----------------------------------------------------------------------------
def _host_prep(x, y, diffuse, normal, rough, v, ls, envWeight, idy, idx):
    """Returns (pix_host, sc_per_dev, g_per_dev)."""
    # per-pixel input slab [P, 10, F]
    fields = [v[0], v[1], v[2], normal[0], normal[1], normal[2],
              diffuse[0], diffuse[1], diffuse[2], rough[0]]
    pixh = np.stack([f.reshape(P, F) for f in
                     [np.asarray(a, np.float32).reshape(NPIX) for a in fields]],
                    axis=1)  # [P, 10, F]
    pixh = np.ascontiguousarray(pixh, dtype=np.float32)

    lsx = np.asarray(ls[0, :, 0], np.float64)
    lsy = np.asarray(ls[0, :, 1], np.float64)
    lsz = np.asarray(ls[0, :, 2], np.float64)
    ew = np.asarray(envWeight[0, :, 0], np.float64)
    ndl = np.clip(lsz, 0.0, 1.0)
    consts = np.zeros((8, S), np.float32)
    consts[0] = lsx
    consts[1] = lsy
    consts[2] = lsz
    consts[3] = lsx ** 2
    consts[4] = 1.0 + lsy ** 2 + lsz ** 2
    consts[5] = ew * ndl                # alpha
    consts[6] = 10.0 * ew * ndl         # beta

    # gathered envmaps: envs [4, EXH*EXW, 3]
    envs = np.concatenate([np.asarray(x, np.float32).reshape(2, EXH * EXW, 3),
                           np.asarray(y, np.float32).reshape(2, EXH * EXW, 3)],
                          axis=0)
    t = (np.asarray(idy, np.int64) * EXW + np.asarray(idx, np.int64)) \
        .reshape(S, NPIX)

    sc_per_dev, g_per_dev = [], []
    for d in range(NDEV):
        sl = slice(SLOC * d, SLOC * (d + 1))
        scd = np.ascontiguousarray(
            np.broadcast_to(consts[:, sl][None, :, :], (P, 8, SLOC)),
            dtype=np.float32)
        td = t[sl]                                   # [SLOC, NPIX]
        g = envs[:, td, :]                           # [4, SLOC, NPIX, 3]
        g = np.transpose(g, (0, 3, 1, 2)).reshape(NCH, SLOC, P, F)
        g = np.transpose(g, (0, 2, 1, 3))            # [NCH, P, SLOC, F]
        g_per_dev.append(np.ascontiguousarray(g).astype(ml_dtypes.bfloat16))
        sc_per_dev.append(scd)
    return pixh, sc_per_dev, g_per_dev


def kernel(x, y, diffuse, normal, rough, seg, v, ls, envWeight, idy, idx):
    from concourse.bass_utils import run_bass_kernel_spmd

    nc, _ = _build()
    pixh, sc_per_dev, g_per_dev = _host_prep(
        x, y, diffuse, normal, rough, v, ls, envWeight, idy, idx)

    in_maps = [{"pix": pixh, "sc": sc_per_dev[d], "g": g_per_dev[d]}
               for d in range(NDEV)]
    res = run_bass_kernel_spmd(nc, in_maps, core_ids=list(range(NDEV)),
                               trace=False)

    total = np.zeros((NCH, P, F), np.float64)
    for d in range(NDEV):
        total += res.results[d]["out"].astype(np.float64)
    fields = total.reshape(4, 3, IM, IM).astype(np.float32)
    pred = fields[0:2]          # envs from x
    gt = fields[2:4]            # envs from y
    pixel_num = float(np.asarray(seg, np.float64).sum()) * pred.shape[0] * 3
    diff = pred.astype(np.float64) - gt.astype(np.float64)
    loss = np.float32((diff ** 2).sum() / pixel_num)
    return (loss, pred[0], gt[0])


# revision 14
# speedup vs baseline: 1.2212x; 1.1840x over previous
"""EnvMap BRDF render-loss kernel for 8 Trainium2 NeuronCores.

Strategy (matches the sharding hint): shard the S=128 sample axis across the
8 cores (16 samples each).  The envmap gather (env[:, idy, idx, :] for the 4
environment images x0,x1,y0,y1) is performed host-side and the gathered
tensors are sharded along S; each core streams its [12 env-channels, 16
samples, 65536 pixels] bf16 slab and computes the full per-(sample,pixel)
BRDF chain + weighted accumulation, producing partial sums [12, 65536] f32.
Host reduces partials over cores, forms pred/gt and the scalar loss.

All math is algebraically identical to the reference:
  camy = normalize(up - (up.n)n);  cross(camy,n) = cross(up,n)/|cy| =
  (nz,0,-nx)/|cy|  =>  camx = (-nz,0,nx)/(|nz|+|nx|)  (L1-normalized).
  n.l = ls_z (basis orthogonality), v.v = 1, |camy| = 1.
  out_{e,c}[pix] = sum_s (alpha_s*diffB_c + beta_s*spec_{s,pix}) * G_{e,c,s,pix}
  with alpha_s = ew_s*ls_z_s, beta_s = 10*ew_s*ls_z_s.
"""

import numpy as np
import ml_dtypes

IM = 256
NPIX = IM * IM          # 65536
EXH, EXW = 128, 256
S = 128
NDEV = 8
SLOC = S // NDEV        # 16 samples per device
P = 128                 # partitions
F = NPIX // P           # 512 free-dim pixels
F0C = 0.05
NCH = 12                # 4 envs x 3 channels
PI = float(np.pi)

_BUILT = None           # cached (nc, meta)


# ----------------------------------------------------------------------------
# Dependency tracker: records ops per engine with explicit semaphore waits.
# ----------------------------------------------------------------------------
class Prog:
    def __init__(self):
        self.ops = {"V": [], "A": [], "S": []}   # per-engine [(waits, emit_fn)]
        self.tick = {"V": 0, "A": 0}
        self.lane_cnt = {}                        # lane name -> dma count
        self.seen = {}                            # (consumer, producer) -> val
        self.buf = {}                             # name -> dict(w=token, r=[tokens])

    def _tok_wait(self, waits, consumer, tok):
        if tok is None:
            return
        prod, val = tok
        if prod == consumer:
            return
        if isinstance(prod, tuple):
            # DMA-lane completion sems can interleave across in-flight DMAs;
            # a wait is only sound for the lane's full issued-so-far count
            # (callers guarantee no later DMA on the lane is issued before
            # this wait passes at runtime).
            val = 16 * self.lane_cnt[prod[1]]
        key = (consumer, prod)
        if self.seen.get(key, 0) >= val:
            return
        self.seen[key] = val
        waits[prod] = max(waits.get(prod, 0), val)

    def op(self, engine, emit_fn, reads=(), writes=(), lane=None):
        waits = {}
        for nm in reads:
            b = self.buf.get(nm)
            if b:
                self._tok_wait(waits, engine, b["w"])
        for nm in writes:
            b = self.buf.get(nm)
            if b:
                self._tok_wait(waits, engine, b["w"])      # WAW
                for t in b["r"]:                           # WAR
                    self._tok_wait(waits, engine, t)
        if engine in ("V", "A"):
            self.tick[engine] += 1
            tok = (engine, self.tick[engine])
        else:
            assert lane is not None
            self.lane_cnt[lane] = self.lane_cnt.get(lane, 0) + 1
            tok = (("L", lane), 16 * self.lane_cnt[lane])
        for nm in writes:
            self.buf[nm] = {"w": tok, "r": []}
        for nm in reads:
            b = self.buf.setdefault(nm, {"w": None, "r": []})
            b["r"].append(tok)
        self.ops[engine].append((waits, emit_fn, tok, lane))


def _build():
    """Build the Bass program once.  Returns (nc, None)."""
    global _BUILT
    if _BUILT is not None:
        return _BUILT
    from contextlib import ExitStack
    import concourse.bass as bass
    from concourse import mybir

    dt = mybir.dt
    Alu = mybir.AluOpType
    Act = mybir.ActivationFunctionType

    nc = bass.Bass("TRN2", target_bir_lowering=False, debug=False,
                   num_devices=NDEV)

    # ---------------- DRAM I/O ----------------
    pix_d = nc.dram_tensor("pix", [P, 10, F], dt.float32, kind="ExternalInput")
    sc_d = nc.dram_tensor("sc", [P, 8, SLOC], dt.float32, kind="ExternalInput")
    g_d = nc.dram_tensor("g", [NCH, P, SLOC, F], dt.bfloat16,
                         kind="ExternalInput")
    out_d = nc.dram_tensor("out", [NCH, P, F], dt.float32,
                           kind="ExternalOutput")
    import os
    dbg = bool(int(os.environ.get("KDEBUG", "0")))
    if dbg:
        wdbg_d = nc.dram_tensor("wdbg", [P, 3, SLOC, F], dt.bfloat16,
                                kind="ExternalOutput")

    ctx = ExitStack()
    sb = lambda name, shape, dtype: ctx.enter_context(
        nc.sbuf_tensor(name, shape, dtype))

    PIX = sb("sPIX", [P, 10, F], dt.float32)
    SC = sb("sSC", [P, 8, SLOC], dt.float32)
    FLD = sb("sFLD", [P, 12, F], dt.float32)
    T = sb("sT", [P, 10, F], dt.float32)      # scratch t0..t9
    C = sb("sC", [P, 9, F], dt.float32)       # chain scratch c0..c8
    W = sb("sW", [P, 3, SLOC, F], dt.bfloat16)
    GT = sb("sGT", [P, 2, SLOC, F], dt.bfloat16)
    PR = sb("sPR", [P, SLOC, F], dt.bfloat16)
    OUTS = sb("sOUTS", [P, NCH, F], dt.float32)
    CB = sb("sCB", [P, 3], dt.float32)        # bias constants

    sem_v = ctx.enter_context(nc.semaphore("sem_v"))
    sem_a = ctx.enter_context(nc.semaphore("sem_a"))
    lane_sems = {}
    for ln in ["in"] + [f"g{j}" for j in range(NCH)] + ["out"]:
        lane_sems[ln] = ctx.enter_context(nc.semaphore(f"lane_{ln}"))

    pr = Prog()
    V, A = nc.vector, nc.scalar

    # helper closures -------------------------------------------------------
    def vop(emit, reads, writes):
        pr.op("V", emit, reads, writes)

    def aop(emit, reads, writes):
        pr.op("A", emit, reads, writes)

    def sop(emit, reads, writes, lane):
        pr.op("S", emit, reads, writes, lane=lane)

    # AP shorthands
    def pix(i):
        return PIX[:, i, :]
    def fld(i):
        return FLD[:, i, :]
    def t(i):
        return T[:, i, :]
    def c(i):
        return C[:, i, :]

    VXi, VYi, VZi, NXi, NYi, NZi, DF0i, DF1i, DF2i, RGi = range(10)
    # field slots
    iVCX, iVCY, iVCZ, iCXX, iDB0, iDB1, iDB2, iA2, iA2M1, iOMK, iKK, iPC = range(12)

    # ---------------- Phase 0: loads ----------------
    sop(lambda: nc.sync.dma_start(PIX[:], pix_d[:]).then_inc(lane_sems["in"], 16),
        [], ["PIX"], lane="in")
    sop(lambda: nc.sync.dma_start(SC[:], sc_d[:]).then_inc(lane_sems["in"], 16),
        [], ["SC"], lane="in")
    for j in range(2):   # prefetch first two G tiles
        def em(j=j):
            return nc.sync.dma_start(GT[:, j % 2, :, :], g_d[j, :, :, :]) \
                .then_inc(lane_sems[f"g{j}"], 16)
        sop(em, [], [f"GT{j % 2}"], lane=f"g{j}")

    # ---------------- Phase 1: per-pixel precompute ----------------
    vop(lambda: V.memset(CB[:, 0:1], -3.49158), [], ["CB"])
    vop(lambda: V.memset(CB[:, 1:2], 0.05), ["CB"], ["CB"])
    vop(lambda: V.memset(CB[:, 2:3], 1.5), ["CB"], ["CB"])
    # cy = up - (up.n) n ; up=(0,1,0)
    vop(lambda: V.scalar_tensor_tensor(t(0), pix(NXi), -1.0, pix(NYi),
                                       Alu.mult, Alu.mult),
        ["PIX"], ["t0"])                                            # cyx
    vop(lambda: V.scalar_tensor_tensor(t(1), pix(NZi), -1.0, pix(NYi),
                                       Alu.mult, Alu.mult),
        ["PIX"], ["t1"])                                            # cyz
    vop(lambda: V.scalar_tensor_tensor(t(2), pix(NYi), -1.0, pix(NYi),
                                       Alu.mult, Alu.mult),
        ["PIX"], ["t2"])                                            # -ny^2
    aop(lambda: A.activation(t(2), t(2), Act.Identity, bias=1.0),
        ["t2"], ["t2"])                                             # cyy
    aop(lambda: A.activation(t(3), t(0), Act.Square), ["t0"], ["t3"])
    aop(lambda: A.activation(t(6), t(1), Act.Square), ["t1"], ["t6"])
    vop(lambda: V.tensor_mul(t(4), t(2), t(2)), ["t2"], ["t4"])
    vop(lambda: V.tensor_add(t(5), t(3), t(4)), ["t3", "t4"], ["t5"])
    vop(lambda: V.tensor_add(t(7), t(5), t(6)), ["t5", "t6"], ["t7"])  # cc
    vop(lambda: V.tensor_scalar(t(8), t(7), 1e-24, None, Alu.max),
        ["t7"], ["t8"])
    aop(lambda: A.activation(t(9), t(8), Act.Ln), ["t8"], ["t9"])
    aop(lambda: A.activation(t(9), t(9), Act.Exp, scale=-0.5),
        ["t9"], ["t9"])  # icn ~= rsqrt(max(cc,1e-24))
    # Newton step for rsqrt accuracy (PWP-table error would otherwise leak
    # into vdl/uu and get amplified by the nom0 cancellation downstream)
    vop(lambda: V.tensor_mul(t(3), t(9), t(9)), ["t9"], ["t3"])
    vop(lambda: V.tensor_mul(t(3), t(3), t(8)), ["t3", "t8"], ["t3"])
    vop(lambda: V.tensor_scalar(t(3), t(3), -0.5, 1.5, Alu.mult, Alu.add),
        ["t3"], ["t3"])
    vop(lambda: V.tensor_mul(t(9), t(9), t(3)), ["t9", "t3"], ["t9"])
    # camy = cy * icn  -> t3,t4,t5
    vop(lambda: V.tensor_mul(t(3), t(0), t(9)), ["t0", "t9"], ["t3"])
    vop(lambda: V.tensor_mul(t(4), t(2), t(9)), ["t2", "t9"], ["t4"])
    vop(lambda: V.tensor_mul(t(5), t(1), t(9)), ["t1", "t9"], ["t5"])
    # vcy = v . camy
    vop(lambda: V.tensor_mul(t(6), pix(VXi), t(3)), ["PIX", "t3"], ["t6"])
    vop(lambda: V.tensor_mul(t(7), pix(VYi), t(4)), ["PIX", "t4"], ["t7"])
    vop(lambda: V.tensor_add(t(6), t(6), t(7)), ["t6", "t7"], ["t6"])
    vop(lambda: V.tensor_mul(t(7), pix(VZi), t(5)), ["PIX", "t5"], ["t7"])
    vop(lambda: V.tensor_add(fld(iVCY), t(6), t(7)), ["t6", "t7"], ["FVCY"])
    # d1 = |nz| + |nx| ; id1
    aop(lambda: A.activation(t(3), pix(NZi), Act.Abs), ["PIX"], ["t3"])
    aop(lambda: A.activation(t(4), pix(NXi), Act.Abs), ["PIX"], ["t4"])
    vop(lambda: V.tensor_add(t(5), t(3), t(4)), ["t3", "t4"], ["t5"])
    vop(lambda: V.tensor_scalar(t(5), t(5), 1e-30, None, Alu.max),
        ["t5"], ["t5"])
    aop(lambda: A.activation(t(8), t(5), Act.Ln), ["t5"], ["t8"])
    aop(lambda: A.activation(t(8), t(8), Act.Exp, scale=-1.0),
        ["t8"], ["t8"])  # id1 ~= 1/(|nz|+|nx|)
    # Newton step r' = r*(2 - x*r)
    vop(lambda: V.tensor_mul(t(9), t(8), t(5)), ["t8", "t5"], ["t9"])
    vop(lambda: V.tensor_scalar(t(9), t(9), -1.0, 2.0, Alu.mult, Alu.add),
        ["t9"], ["t9"])
    vop(lambda: V.tensor_mul(t(8), t(8), t(9)), ["t8", "t9"], ["t8"])
    # vcx = (-vx*nz + vz*nx) * id1
    vop(lambda: V.scalar_tensor_tensor(t(6), pix(VXi), -1.0, pix(NZi),
                                       Alu.mult, Alu.mult),
        ["PIX"], ["t6"])
    vop(lambda: V.tensor_mul(t(7), pix(VZi), pix(NXi)), ["PIX"], ["t7"])
    vop(lambda: V.tensor_add(t(6), t(6), t(7)), ["t6", "t7"], ["t6"])
    vop(lambda: V.tensor_mul(fld(iVCX), t(6), t(8)), ["t6", "t8"], ["FVCX"])
    # cxx = (nz^2 + nx^2) * id1^2
    aop(lambda: A.activation(t(6), pix(NZi), Act.Square), ["PIX"], ["t6"])
    aop(lambda: A.activation(t(7), pix(NXi), Act.Square), ["PIX"], ["t7"])
    vop(lambda: V.tensor_add(t(6), t(6), t(7)), ["t6", "t7"], ["t6"])
    vop(lambda: V.tensor_mul(t(7), t(8), t(8)), ["t8"], ["t7"])
    vop(lambda: V.tensor_mul(fld(iCXX), t(6), t(7)), ["t6", "t7"], ["FCXX"])
    # vcz = v . n
    vop(lambda: V.tensor_mul(t(6), pix(VXi), pix(NXi)), ["PIX"], ["t6"])
    vop(lambda: V.tensor_mul(t(7), pix(VYi), pix(NYi)), ["PIX"], ["t7"])
    vop(lambda: V.tensor_add(t(6), t(6), t(7)), ["t6", "t7"], ["t6"])
    vop(lambda: V.tensor_mul(t(7), pix(VZi), pix(NZi)), ["PIX"], ["t7"])
    vop(lambda: V.tensor_add(fld(iVCZ), t(6), t(7)), ["t6", "t7"], ["FVCZ"])
    # diffB_c = (df + 1) / (2*pi)
    for cch in range(3):
        def em(cch=cch):
            return V.tensor_scalar(fld(iDB0 + cch), pix(DF0i + cch),
                                   1.0 / (2 * PI), 1.0 / (2 * PI),
                                   Alu.mult, Alu.add)
        vop(em, ["PIX"], [f"FDB{cch}"])
    # rough path
    vop(lambda: V.tensor_scalar(t(6), pix(RGi), 0.5, 0.5, Alu.mult, Alu.add),
        ["PIX"], ["t6"])                                            # roughB
    aop(lambda: A.activation(t(7), t(6), Act.Square, bias=1.0), ["t6"], ["t7"])
    vop(lambda: V.tensor_scalar(t(9), t(7), 0.125, None, Alu.mult),
        ["t7"], ["t9"])                                             # kk
    vop(lambda: V.tensor_scalar(t(5), t(7), -0.125, 1.0, Alu.mult, Alu.add),
        ["t7"], ["t5"])                                             # omk
    aop(lambda: A.activation(t(8), t(6), Act.Square), ["t6"], ["t8"])
    aop(lambda: A.activation(fld(iA2), t(8), Act.Square), ["t8"], ["FA2"])
    vop(lambda: V.tensor_scalar(fld(iA2M1), fld(iA2), -1.0, None, Alu.add),
        ["FA2"], ["FA2M1"])
    # pc = 4*pi*(ndv*(1-k)+k),  ndv = clip(vcz,0,1)
    vop(lambda: V.tensor_scalar(t(6), fld(iVCZ), 0.0, 1.0, Alu.max, Alu.min),
        ["FVCZ"], ["t6"])
    vop(lambda: V.tensor_mul(t(7), t(6), t(5)), ["t6", "t5"], ["t7"])
    vop(lambda: V.tensor_add(t(7), t(7), t(9)), ["t7", "t9"], ["t7"])
    vop(lambda: V.tensor_scalar(t(8), t(7), 4 * PI, None, Alu.mult),
        ["t7"], ["t8"])                                             # pc
    vop(lambda: V.tensor_mul(fld(iOMK), t(8), t(5)), ["t8", "t5"], ["FOMK"])
    vop(lambda: V.tensor_mul(fld(iKK), t(8), t(9)), ["t8", "t9"], ["FKK"])

    # ---------------- Phase 2: per-sample chain ----------------
    # Two samples are software-pipelined with independent scratch banks so
    # the V<->A dependency chain of one sample overlaps the other's work.
    LN2 = float(np.log(2.0))

    def chain_ops(s, bank):
        """Yield (engine, emit, reads, writes) for one sample's chain."""
        BUF = C if bank == 0 else T

        def c(i):
            return BUF[:, i, :]

        def n(i):
            return f"b{bank}_c{i}"

        X = SC[:, 0, s:s + 1]
        Y = SC[:, 1, s:s + 1]
        Z = SC[:, 2, s:s + 1]
        X2 = SC[:, 3, s:s + 1]
        CCS = SC[:, 4, s:s + 1]
        AL = SC[:, 5, s:s + 1]
        BE = SC[:, 6, s:s + 1]

        yield ("A", lambda: A.activation(c(0), fld(iVCZ), Act.Copy, scale=Z),
               ["FVCZ", "SC"], [n(0)])
        yield ("V", lambda: V.scalar_tensor_tensor(c(1), fld(iVCY), Y, c(0),
                                                   Alu.mult, Alu.add),
               ["FVCY", n(0), "SC"], [n(1)])
        yield ("V", lambda: V.scalar_tensor_tensor(c(0), fld(iVCX), X, c(1),
                                                   Alu.mult, Alu.add),
               ["FVCX", n(1), "SC"], [n(0)])                  # vdl
        yield ("A", lambda: A.activation(c(1), c(0), Act.Identity,
                                         scale=2.0, bias=CCS),
               [n(0), "SC"], [n(1)])
        yield ("V", lambda: V.scalar_tensor_tensor(c(1), fld(iCXX), X2, c(1),
                                                   Alu.mult, Alu.add),
               ["FCXX", n(1), "SC"], [n(1)])                  # uu
        yield ("V", lambda: V.tensor_scalar(c(1), c(1), 4e-6, None, Alu.max),
               [n(1)], [n(1)])
        yield ("A", lambda: A.activation(c(2), c(1), Act.Ln, scale=0.25),
               [n(1)], [n(2)])
        yield ("A", lambda: A.activation(c(3), c(2), Act.Exp, scale=-0.5),
               [n(2)], [n(3)])      # denom seed
        # Newton step d' = d*(1.5 - 0.125*m*d^2): PWP-table error otherwise
        # gets amplified ~1000x by the nom0 cancellation on highlights.
        yield ("V", lambda: V.tensor_mul(c(2), c(3), c(3)), [n(3)], [n(2)])
        yield ("V", lambda: V.tensor_mul(c(2), c(2), c(1)), [n(2), n(1)], [n(2)])
        yield ("A", lambda: A.activation(c(2), c(2), Act.Identity, scale=-0.125,
                                         bias=CB[:, 2:3]),
               [n(2), "CB"], [n(2)])
        yield ("V", lambda: V.tensor_mul(c(3), c(3), c(2)), [n(3), n(2)], [n(3)])
        yield ("V", lambda: V.scalar_tensor_tensor(c(4), c(0), 1.0, c(3),
                                                   Alu.add, Alu.mult),
               [n(0), n(3)], [n(4)])                          # p1
        yield ("A", lambda: A.activation(c(5), c(4), Act.Identity,
                                         scale=-1.38868, bias=CB[:, 0:1]),
               [n(4), "CB"], [n(5)])
        yield ("V", lambda: V.tensor_mul(c(5), c(5), c(4)), [n(5), n(4)], [n(5)])
        yield ("A", lambda: A.activation(c(5), c(5), Act.Exp, scale=LN2),
               [n(5)], [n(5)])
        yield ("A", lambda: A.activation(c(5), c(5), Act.Identity,
                                         scale=1.0 - F0C, bias=CB[:, 1:2]),
               [n(5), "CB"], [n(5)])                          # e1
        yield ("V", lambda: V.scalar_tensor_tensor(c(6), fld(iVCZ), Z, c(3),
                                                   Alu.add, Alu.mult),
               ["FVCZ", n(3), "SC"], [n(6)])                  # ndu*denom
        # ndh = max(0, 0.5*ndu*denom); <=1 clip redundant (Cauchy-Schwarz),
        # rounding excess is harmless since nom0 is squared.
        yield ("A", lambda: A.activation(c(6), c(6), Act.Relu, scale=0.5),
               [n(6)], [n(6)])                                # ndh
        yield ("A", lambda: A.activation(c(7), c(6), Act.Square), [n(6)], [n(7)])
        yield ("V", lambda: V.tensor_mul(c(7), c(7), fld(iA2M1)),
               [n(7), "FA2M1"], [n(7)])
        yield ("A", lambda: A.activation(c(7), c(7), Act.Square, bias=1.0),
               [n(7)], [n(7)])                                # nom0^2
        yield ("V", lambda: V.scalar_tensor_tensor(c(8), fld(iOMK), Z, fld(iKK),
                                                   Alu.mult, Alu.add),
               ["FOMK", "FKK", "SC"], [n(8)])                 # pc*nom2
        yield ("V", lambda: V.tensor_mul(c(7), c(7), c(8)), [n(7), n(8)], [n(7)])
        yield ("V", lambda: V.tensor_scalar(c(7), c(7), 1e-6, None, Alu.max),
               [n(7)], [n(7)])
        yield ("A", lambda: A.activation(c(7), c(7), Act.Ln), [n(7)], [n(7)])
        yield ("A", lambda: A.activation(c(8), c(7), Act.Exp, scale=-1.0),
               [n(7)], [n(8)])                                # rnom
        yield ("V", lambda: V.tensor_mul(c(8), c(8), c(5)), [n(8), n(5)], [n(8)])
        yield ("V", lambda: V.scalar_tensor_tensor(c(8), fld(iA2), BE, c(8),
                                                   Alu.mult, Alu.mult),
               ["FA2", n(8), "SC"], [n(8)])                   # sw
        yield ("V", lambda: V.scalar_tensor_tensor(
                   W[:, :, s, :], FLD[:, iDB0:iDB0 + 3, :], AL,
                   BUF[:, 8:9, :].to_broadcast([P, 3, F]), Alu.mult, Alu.add),
               ["FDB0", "FDB1", "FDB2", n(8), "SC"],
               [f"W0_{s}", f"W1_{s}", f"W2_{s}"])

    from itertools import zip_longest
    for pair in range(SLOC // 2):
        ops0 = list(chain_ops(2 * pair, 0))
        ops1 = list(chain_ops(2 * pair + 1, 1))
        for o0, o1 in zip_longest(ops0, ops1):
            for o in (o0, o1):
                if o is None:
                    continue
                eng, emit, reads, writes = o
                pr.op(eng, emit, reads, writes)

    if dbg:
        wr_all = [f"W{cc_}_{s_}" for cc_ in range(3) for s_ in range(SLOC)]
        sop(lambda: nc.sync.dma_start(wdbg_d[:], W[:]).then_inc(
            lane_sems["out"], 16), wr_all, [], lane="out")

    # ---------------- Phase 3: MAC over samples per (env,channel) ----------
    for j in range(NCH):
        b = j % 2
        cch = j % 3
        wr = [f"W{cch}_{s}" for s in range(SLOC)]
        vop(lambda j=j, b=b, cch=cch: V.tensor_mul(PR[:], W[:, cch, :, :],
                                                   GT[:, b, :, :]),
            wr + [f"GT{b}"], ["PR"])
        if j + 2 < NCH:
            def em(j=j, b=b):
                return nc.sync.dma_start(GT[:, b, :, :], g_d[j + 2, :, :, :]) \
                    .then_inc(lane_sems[f"g{j + 2}"], 16)
            sop(em, [], [f"GT{b}"], lane=f"g{j + 2}")
        h = SLOC // 2
        while h >= 1:
            if h > 1:
                def em(h=h):
                    return V.tensor_add(PR[:, 0:h, :], PR[:, 0:h, :],
                                        PR[:, h:2 * h, :])
                vop(em, ["PR"], ["PR"])
            else:
                def em(j=j):
                    return V.tensor_add(OUTS[:, j, :], PR[:, 0, :], PR[:, 1, :])
                vop(em, ["PR"], [f"OUTS{j}"])
            h //= 2
        def em(j=j):
            return nc.sync.dma_start(out_d[j, :, :], OUTS[:, j, :]) \
                .then_inc(lane_sems["out"], 16)
        sop(em, [f"OUTS{j}"], [], lane="out")

    # ---------------- Emit ----------------
    sems = {"V": sem_v, "A": sem_a}

    def emit_stream(key, eng):
        for waits, emit_fn, tok, lane in pr.ops[key]:
            for prod, val in waits.items():
                if isinstance(prod, tuple):        # ('L', lane)
                    eng.wait_ge(lane_sems[prod[1]], val)
                else:
                    eng.wait_ge(sems[prod], val)
            ins = emit_fn()
            if key in ("V", "A"):
                ins.then_inc(sems[key], 1)

    with nc.Block() as block:
        @block.sync
        def _(eng):
            emit_stream("S", nc.sync)

        @block.vector
        def _(eng):
            emit_stream("V", nc.vector)

        @block.scalar
        def _(eng):
            emit_stream("A", nc.scalar)

    ctx.close()
    _BUILT = (nc, None)
    return _BUILT


# ----------------------------------------------------------------------------
# Host side
# ----------------------------------------------------------------------------
def _host_prep(x, y, diffuse, normal, rough, v, ls, envWeight, idy, idx):
    """Returns (pix_host, sc_per_dev, g_per_dev)."""
    # per-pixel input slab [P, 10, F]
    fields = [v[0], v[1], v[2], normal[0], normal[1], normal[2],
              diffuse[0], diffuse[1], diffuse[2], rough[0]]
    pixh = np.stack([f.reshape(P, F) for f in
                     [np.asarray(a, np.float32).reshape(NPIX) for a in fields]],
                    axis=1)  # [P, 10, F]
    pixh = np.ascontiguousarray(pixh, dtype=np.float32)

    lsx = np.asarray(ls[0, :, 0], np.float64)
    lsy = np.asarray(ls[0, :, 1], np.float64)
    lsz = np.asarray(ls[0, :, 2], np.float64)
    ew = np.asarray(envWeight[0, :, 0], np.float64)
    ndl = np.clip(lsz, 0.0, 1.0)
    consts = np.zeros((8, S), np.float32)
    consts[0] = lsx
    consts[1] = lsy
    consts[2] = lsz
    consts[3] = lsx ** 2
    consts[4] = 1.0 + lsy ** 2 + lsz ** 2
    consts[5] = ew * ndl                # alpha
    consts[6] = 10.0 * ew * ndl         # beta

    # gathered envmaps: envs [4, EXH*EXW, 3]
    envs = np.concatenate([np.asarray(x, np.float32).reshape(2, EXH * EXW, 3),
                           np.asarray(y, np.float32).reshape(2, EXH * EXW, 3)],
                          axis=0)
    t = (np.asarray(idy, np.int64) * EXW + np.asarray(idx, np.int64)) \
        .reshape(S, NPIX)

    sc_per_dev, g_per_dev = [], []
    for d in range(NDEV):
        sl = slice(SLOC * d, SLOC * (d + 1))
        scd = np.ascontiguousarray(
            np.broadcast_to(consts[:, sl][None, :, :], (P, 8, SLOC)),
            dtype=np.float32)
        td = t[sl]                                   # [SLOC, NPIX]
        g = envs[:, td, :]                           # [4, SLOC, NPIX, 3]
        g = np.transpose(g, (0, 3, 1, 2)).reshape(NCH, SLOC, P, F)
        g = np.transpose(g, (0, 2, 1, 3))            # [NCH, P, SLOC, F]
        g_per_dev.append(np.ascontiguousarray(g).astype(ml_dtypes.bfloat16))
        sc_per_dev.append(scd)
    return pixh, sc_per_dev, g_per_dev


def kernel(x, y, diffuse, normal, rough, seg, v, ls, envWeight, idy, idx):
    from concourse.bass_utils import run_bass_kernel_spmd

    nc, _ = _build()
    pixh, sc_per_dev, g_per_dev = _host_prep(
        x, y, diffuse, normal, rough, v, ls, envWeight, idy, idx)

    in_maps = [{"pix": pixh, "sc": sc_per_dev[d], "g": g_per_dev[d]}
               for d in range(NDEV)]
    res = run_bass_kernel_spmd(nc, in_maps, core_ids=list(range(NDEV)),
                               trace=False)

    total = np.zeros((NCH, P, F), np.float64)
    for d in range(NDEV):
        total += res.results[d]["out"].astype(np.float64)
    fields = total.reshape(4, 3, IM, IM).astype(np.float32)
    pred = fields[0:2]          # envs from x
    gt = fields[2:4]            # envs from y
    pixel_num = float(np.asarray(seg, np.float64).sum()) * pred.shape[0] * 3
    diff = pred.astype(np.float64) - gt.astype(np.float64)
    loss = np.float32((diff ** 2).sum() / pixel_num)
    return (loss, pred[0], gt[0])
